# revision 33
# baseline (speedup 1.0000x reference)
"""AttnBlock (GroupNorm + 1x1-conv QKV + single-head spatial attention + proj
+ residual) on 8 Trainium2 NeuronCores.

Sharding: pure data-parallel over batch — 16 samples / 8 cores = 2 samples per
core; weights broadcast. No collectives; gather on host.

Key optimizations over the 133.6us two-GEMM version:
  - q/k GEMMs replaced by ONE GEMM: softmax is over j, so
    scores = (Wq h)^T (Wk h) = h^T (Wq^T Wk) h = h^T g with g = A h and
    A = Wq^T Wk precomputed on host (f64). Removes 1/7 of the PE work
    and half the QK PSUM->SBUF copies. The q-bias cross term
    (Wk^T cq)^T h is exactly zero here (norm_b = qkv_b = 0; asserted).
  - 1/S broadcast via a K=1 ones-matmul on the PE (~0.2us) instead of
    the DRAM bounce (~6us of exposed HBM round-trip latency).
  - head: bn_stats run per 512KB x chunk as it lands; filler matmuls on
    a memset tile keep the PE HAM-warm through the DMA window so the
    body runs at 2.4GHz from the first real MM. Every strict-FIFO
    engine queue is emitted in expected-ready order (anything else
    head-of-line blocks the queue on a DMA).
  - gn1 emitted between pv0 and scores0 with its DVE stats interleaved
    into the pv0-evac stream, so its Sqrt lands on ACT before the
    exp0 burst: 3 ACT table loads total (exp, sqrt, exp), all hidden.
  - o-group PSUMs alternate between two pools (7 banks effective) so
    the o1 evacs (gated on the S1 chain) never starve the PE of banks;
    the first two o1 groups are emitted before the S1 denominator
    matmuls to cover the e1 add-tree latency.
  - all matmul operands bf16 (1 cycle/row), PSUM accumulation fp32;
    proj folded into the v weights (W_pv = proj_w @ W_v) with its bias
    riding the vT tiles; norm affine folded into the QKV weights.

Per-core formulation (per sample, C=512 channels, N=1024 spatial):
  xn  = (x - mu_g) * rstd_g        (bn_stats + tiny indicator matmuls)
  g   = A' xn                      (C on partitions, spatial free)
  vT  = xn^T Wpv'^T + b_out        (spatial on partitions, C free)
  e   = exp(xn^T g * C^-0.5)       (logits O(1); no max-subtraction)
  S   = ones^T tree(e)
  y   = x + (vT^T e) * (1/S)
"""

import numpy as np
import ml_dtypes
_BF16 = ml_dtypes.bfloat16

import concourse.bass as bass
import concourse.tile as tile
from concourse import bacc, mybir
from concourse.bass_utils import run_bass_kernel_spmd

B, C, H, W = 16, 512, 32, 32
N = H * W              # 1024 spatial positions
G = 32                 # groups
GS = C // G            # 16 channels per group
NCORES = 8
SPC = B // NCORES      # samples per core
EPS = 1e-6
SCALE = float(C) ** -0.5
KT = C // 128          # 4 channel tiles of 128
NT = N // 128          # 8 spatial tiles of 128
NH = N // 512          # 2 free-dim halves of 512

F32 = mybir.dt.float32
F32R = mybir.dt.float32r
F16 = mybir.dt.bfloat16

_BUILD_CACHE = {}
LAST_RESULT = None  # BassKernelResults of the most recent run (for test harness)


def _build():
    nc = bacc.Bacc("TRN2", target_bir_lowering=False, debug=False)

    x_ext = nc.declare_dram_parameter("x", [SPC, C, N], F32, isOutput=False)
    waT_ext = nc.declare_dram_parameter("waT", [C, C], F16, isOutput=False)
    wpvT_ext = nc.declare_dram_parameter("wpvT", [C, C], F16, isOutput=False)
    boutbc_ext = nc.declare_dram_parameter("bout_bc", [128, C], F16, isOutput=False)
    ind_ext = nc.declare_dram_parameter("ind16", [128, 8], F32, isOutput=False)
    indT_ext = nc.declare_dram_parameter("ind16T", [8, 128], F32, isOutput=False)
    ones_ext = nc.declare_dram_parameter("ones16", [128, 1], F16, isOutput=False)
    y_ext = nc.declare_dram_parameter("y", [SPC, C, N], F32, isOutput=True)

    Identity = mybir.ActivationFunctionType.Identity
    Copy = mybir.ActivationFunctionType.Copy
    Exp = mybir.ActivationFunctionType.Exp
    Sqrt = mybir.ActivationFunctionType.Sqrt
    mult = mybir.AluOpType.mult
    add = mybir.AluOpType.add

    with tile.TileContext(nc) as tc:
        with (
            tc.tile_pool(name="wpool", bufs=1) as wpool,
            tc.tile_pool(name="cpool", bufs=1) as cpool,
            tc.tile_pool(name="xpool", bufs=2) as xpool,
            tc.tile_pool(name="hpool", bufs=1) as hpool,
            tc.tile_pool(name="gpool", bufs=1) as gpool,
            tc.tile_pool(name="vpool", bufs=1) as vpool,
            tc.tile_pool(name="epool", bufs=1) as epool,
            tc.tile_pool(name="opool", bufs=4) as opool,
            tc.tile_pool(name="gnpool", bufs=2) as gnpool,
            tc.tile_pool(name="spool", bufs=1) as spool,
            tc.tile_pool(name="psA", bufs=4, space="PSUM") as psA,
            tc.tile_pool(name="ps", bufs=3, space="PSUM") as ps,
            tc.tile_pool(name="psg", bufs=1, space="PSUM") as psg,
        ):
            # ---- DMA plan (proven baseline ring layout): x0 first on both
            # HWDGE rings, small F32R constants on sync, then x1, then the
            # A weights on sync; wpv + remaining constants on SWDGE. ----
            x_tiles = [
                xpool.tile([128, KT, N], F32, tag="x", name=f"x_sb{s}")
                for s in range(SPC)
            ]
            # x0 in 256KB half-chunks so the per-transfer completion
            # semaphores pipeline with the bn_stats consumers
            for kt in range(KT):
                eng = nc.sync if kt % 2 == 0 else nc.scalar
                for sg in range(2):
                    eng.dma_start(
                        out=x_tiles[0][:, kt, sg * 512 : (sg + 1) * 512],
                        in_=x_ext.ap()[
                            0, kt * 128 : (kt + 1) * 128, sg * 512 : (sg + 1) * 512
                        ],
                    )
            ind_sb = cpool.tile([128, 8], F32)
            nc.sync.dma_start(out=ind_sb, in_=ind_ext.ap())
            indT_sb = cpool.tile([8, 128], F32)
            nc.sync.dma_start(out=indT_sb, in_=indT_ext.ap())

            # Everything below is floored: queued transfers round-robin
            # with x0's packets at the SDMA level (different logical
            # queues), so an early issue steals a share of the pipe from
            # the latency-critical x0 stream.
            waT_sb = wpool.tile([128, KT, C], F16)
            wpv_sb = wpool.tile([128, KT, C], F16)
            with tc.tile_wait_until(0.016):
                for kt in range(KT):
                    nc.sync.dma_start(
                        out=waT_sb[:, kt, :],
                        in_=waT_ext.ap()[kt * 128 : (kt + 1) * 128, :],
                    )
                for kt in range(KT):
                    nc.scalar.dma_start(
                        out=wpv_sb[:, kt, :],
                        in_=wpvT_ext.ap()[kt * 128 : (kt + 1) * 128, :],
                    )
            with tc.tile_wait_until(0.018):
                for kt in range(KT):
                    eng = nc.scalar if kt % 2 == 0 else nc.sync
                    eng.dma_start(
                        out=x_tiles[1][:, kt, :],
                        in_=x_ext.ap()[1, kt * 128 : (kt + 1) * 128, :],
                    )
            bout_bc = cpool.tile([128, C], F16)
            nc.gpsimd.dma_start(out=bout_bc, in_=boutbc_ext.ap())
            ones_col = cpool.tile([128, 1], F16)
            nc.gpsimd.dma_start(out=ones_col, in_=ones_ext.ap())

            eps_sb = cpool.tile([8, 1], F32)
            nc.vector.memset(eps_sb, EPS)
            ones_rf = cpool.tile([1, 128], F32)
            nc.vector.memset(ones_rf, 1.0)
            ones_row = cpool.tile([1, 128], F32R)
            nc.vector.tensor_copy(ones_row, ones_rf)
            warm_sb = cpool.tile([8, 1], F32)
            # ACT table warmups: Exp first, Sqrt second, so the sqrt set
            # is resident for both gn chains; one hidden reload to exp at
            # the start of the scores0 burst.
            nc.scalar.activation(out=warm_sb, in_=eps_sb, func=Exp)
            nc.scalar.activation(out=warm_sb, in_=eps_sb, func=Sqrt)

            # ---- PE warmup fillers: keep HAM busy ~8->18us. ----
            warm_mm = cpool.tile([128, 512], F16)
            nc.vector.memset(warm_mm, 0.0)
            _fill_n = [0]

            def fill(n):
                """HAM-keepalive filler matmuls (memset operand, dead PSUM)."""
                for _ in range(n):
                    i = _fill_n[0]
                    _fill_n[0] += 1
                    pw = ps.tile([128, 512], F32, tag="mm", name=f"warm{i}")
                    nc.tensor.matmul(
                        pw, warm_mm[:, 0:128], warm_mm, start=True, stop=True
                    )

            fill(38)

            h_tiles = [
                hpool.tile([128, KT, N], F16, tag=f"h{s}", name=f"h{s}")
                for s in range(SPC)
            ]

            def gn_stats(s):
                """bn_stats per chunk (DVE, paced by the x DMA); batched
                slab chain -> mr (8,KT,2) [-mean*rstd, rstd]."""
                x_sb = x_tiles[s]
                mv = gnpool.tile([128, KT, 2], F32, tag="mv", name=f"mv{s}")
                stat_tiles = []
                for kt in range(KT):
                    stats = gnpool.tile(
                        [128, 2, 6], F32, tag=f"stats{kt}", name=f"stats{s}_{kt}"
                    )
                    stat_tiles.append(stats)
                    for sg in range(2):
                        nc.vector.bn_stats(
                            out=stats[:, sg, :],
                            in_=x_sb[:, kt, sg * 512 : (sg + 1) * 512],
                        )
                for kt in range(KT):
                    nc.vector.bn_aggr(out=mv[:, kt, :], in_=stat_tiles[kt])
                return gn_chain(s, mv, nc.vector)

            def gn_chain(s, mv, eng):
                """eng handles the pure-SBUF slab algebra (nc.vector for
                sample 0 where DVE is free; nc.gpsimd for sample 1 so the
                chain doesn't contend with the pv0/scores0 DVE evacs).
                PSUM reads and the reciprocal stay on DVE.  The indicator
                matmuls run in plain fp32 (tiny free dims) so no F32R
                casts are needed and mv feeds the matmul directly."""
                # in place: mv[...,1] = E[x^2] = var + mean^2
                tmp = gnpool.tile([128, KT], F32, tag="tmp", name=f"tmp{s}")
                eng.tensor_mul(tmp, mv[:, :, 0], mv[:, :, 0])
                eng.tensor_add(mv[:, :, 1], mv[:, :, 1], tmp)
                ps_gs = psg.tile([8, KT, 2], F32, tag="gn", name=f"ps_gs{s}")
                for kt in range(KT):
                    nc.tensor.matmul(
                        ps_gs[:, kt, :], ind_sb, mv[:, kt, :], start=True, stop=True
                    )
                if s == 0:
                    fill(6)  # keep HAM warm while the tiny chain computes
                gs = gnpool.tile([8, KT, 2], F32, tag="gs", name=f"gs{s}")
                nc.vector.tensor_scalar_mul(gs, ps_gs, 1.0 / GS)
                msq = gnpool.tile([8, KT], F32, tag="msq", name=f"msq{s}")
                eng.tensor_mul(msq, gs[:, :, 0], gs[:, :, 0])
                eng.tensor_sub(gs[:, :, 1], gs[:, :, 1], msq)
                nc.scalar.activation(
                    out=gs[:, :, 1], in_=gs[:, :, 1], func=Sqrt, bias=eps_sb
                )
                nc.vector.reciprocal(gs[:, :, 1], gs[:, :, 1])
                eng.tensor_mul(gs[:, :, 0], gs[:, :, 0], gs[:, :, 1])
                eng.tensor_scalar_mul(gs[:, :, 0], gs[:, :, 0], -1.0)
                return gs

            def gn_apply(s, mr, engines):
                """Broadcast stats to channels (one matmul for all kt), apply
                (x-mu)*rstd -> h bf16."""
                x_sb = x_tiles[s]
                ps_bc = psg.tile([128, KT, 2], F32, tag="gn", name=f"ps_bc{s}")
                nc.tensor.matmul(ps_bc, indT_sb, mr, start=True, stop=True)
                scb = gnpool.tile([128, KT, 2], F32, tag="scb", name=f"scb{s}")
                nc.vector.tensor_copy(scb, ps_bc)
                for kt in range(KT):
                    if engines[kt] == "v":
                        nc.vector.tensor_scalar(
                            out=h_tiles[s][:, kt, :],
                            in0=x_sb[:, kt, :],
                            scalar1=scb[:, kt, 1:2],
                            scalar2=scb[:, kt, 0:1],
                            op0=mult,
                            op1=add,
                        )
                    else:
                        nc.scalar.activation(
                            out=h_tiles[s][:, kt, :], in_=x_sb[:, kt, :],
                            func=Identity, scale=scb[:, kt, 1:2],
                            bias=scb[:, kt, 0:1],
                        )

            def head0(g_sb):
                """Sample-0: gn (batched, DMA-paced bn_stats), apply in
                ih-halves (the g ih0 sweep only needs the first halves),
                then g = A' h kt-inner over the ps rotation."""
                mr0 = gn_stats(0)
                ps_bc = psg.tile([128, KT, 2], F32, tag="gn", name="ps_bc0")
                nc.tensor.matmul(ps_bc, indT_sb, mr0, start=True, stop=True)
                fill(4)
                scb = gnpool.tile([128, KT, 2], F32, tag="scb", name="scb0")
                nc.vector.tensor_copy(scb, ps_bc)
                x_sb = x_tiles[0]
                h_sb = h_tiles[0]

                def apply_half(kt, ih, eng):
                    sl = slice(ih * 512, (ih + 1) * 512)
                    if eng == "v":
                        nc.vector.tensor_scalar(
                            out=h_sb[:, kt, sl], in0=x_sb[:, kt, sl],
                            scalar1=scb[:, kt, 1:2], scalar2=scb[:, kt, 0:1],
                            op0=mult, op1=add,
                        )
                    else:
                        nc.scalar.activation(
                            out=h_sb[:, kt, sl], in_=x_sb[:, kt, sl],
                            func=Identity, scale=scb[:, kt, 1:2],
                            bias=scb[:, kt, 0:1],
                        )

                # ih0: kt-OUTER over 4 psA banks so the g matmul cadence
                # (4 MMs ~0.86us per kt) pipelines with the apply cadence
                # (one per-kt half-apply every ~0.5-0.8us, v/a split)
                for kt in range(KT):
                    apply_half(kt, 0, "vava"[kt])
                pms0 = [
                    psA.tile([128, 512], F32, tag="gA", name=f"g0h_{ot}")
                    for ot in range(KT)
                ]
                for kt in range(KT):
                    for ot in range(KT):
                        nc.tensor.matmul(
                            pms0[ot],
                            waT_sb[:, kt, ot * 128 : (ot + 1) * 128],
                            h_sb[:, kt, 0:512],
                            start=(kt == 0),
                            stop=(kt == KT - 1),
                        )
                for kt in range(KT):
                    apply_half(kt, 1, "vava"[kt])
                for ot in range(KT):
                    dest = g_sb[:, ot, 0:512]
                    if ot % 2 == 0:
                        nc.scalar.activation(out=dest, in_=pms0[ot], func=Copy)
                    else:
                        nc.vector.tensor_copy(dest, pms0[ot])
                # ih1: plain kt-inner over the ps rotation (h1 ready)
                for ot in range(KT):
                    pm = ps.tile([128, 512], F32, tag="mm")
                    for kt in range(KT):
                        nc.tensor.matmul(
                            pm,
                            waT_sb[:, kt, ot * 128 : (ot + 1) * 128],
                            h_sb[:, kt, 512:1024],
                            start=(kt == 0),
                            stop=(kt == KT - 1),
                        )
                    dest = g_sb[:, ot, 512:1024]
                    if ot % 2 == 0:
                        nc.scalar.activation(out=dest, in_=pm, func=Copy)
                    else:
                        nc.vector.tensor_copy(dest, pm)

            def gn1_stats_kt(kt, stat_tiles):
                stats = gnpool.tile(
                    [128, 2, 6], F32, tag=f"stats{kt}", name=f"stats1_{kt}"
                )
                stat_tiles.append(stats)
                for sg in range(2):
                    nc.vector.bn_stats(
                        out=stats[:, sg, :],
                        in_=x_tiles[1][:, kt, sg * 512 : (sg + 1) * 512],
                    )

            def gn1_chain(stat_tiles):
                """Batched chain + apply for sample 1; apply on DVE since
                ACT is saturated by exp0."""
                mv = gnpool.tile([128, KT, 2], F32, tag="mv1", name="mv1")
                for kt in range(KT):
                    nc.vector.bn_aggr(out=mv[:, kt, :], in_=stat_tiles[kt])
                mr = gn_chain(1, mv, nc.gpsimd)
                gn_apply(1, mr, "vvvv")

            def pv0(vT_sb, stat1_tiles):
                """vT0 = h0^T Wpv'^T + b_out.  nt0-3 rotate in the ps pool
                (their DVE adds run first so scores0 gets banks on time);
                nt4-7 use the psA banks (their adds are interleaved with
                gn1's DVE stats and may run late — harmless)."""
                h_sb = h_tiles[0]
                pms = {}
                for nt in range(NT):
                    pool = ps if nt < 4 else psA
                    pm = pool.tile([128, 512], F32, tag="mm" if nt < 4 else "gA",
                                   name=f"pv0_{nt}")
                    pms[nt] = pm
                    for kt in range(KT):
                        nc.tensor.matmul(
                            pm,
                            h_sb[:, kt, nt * 128 : (nt + 1) * 128],
                            wpv_sb[:, kt, :],
                            start=(kt == 0),
                            stop=(kt == KT - 1),
                        )
                for nt in range(4):
                    nc.vector.tensor_add(vT_sb[:, nt, :], pms[nt], bout_bc)
                # gn1's DVE stats, floored so the scheduler cannot slot them
                # between gn0's chain ops (strict-FIFO head-of-line risk)
                with tc.tile_wait_until(0.023):
                    for kt in range(KT):
                        gn1_stats_kt(kt, stat1_tiles)
                for nt in range(4, NT):
                    nc.vector.tensor_add(vT_sb[:, nt, :], pms[nt], bout_bc)

            def pv1(vT_sb):
                h_sb = h_tiles[1]
                for nt in range(NT):
                    pm = ps.tile([128, 512], F32, tag="mm")
                    for kt in range(KT):
                        nc.tensor.matmul(
                            pm,
                            h_sb[:, kt, nt * 128 : (nt + 1) * 128],
                            wpv_sb[:, kt, :],
                            start=(kt == 0),
                            stop=(kt == KT - 1),
                        )
                    nc.vector.tensor_add(vT_sb[:, nt, :], pm, bout_bc)

            def scores(s, g_sb, e_sb):
                """e = exp(h^T g * C^-0.5); keys on partitions."""
                h_sb = h_tiles[s]
                for ih in range(NH):
                    for jt in range(NT):
                        pm = ps.tile([128, 512], F32, tag="mm")
                        for ct in range(KT):
                            nc.tensor.matmul(
                                pm,
                                g_sb[:, ct, jt * 128 : (jt + 1) * 128],
                                h_sb[:, ct, ih * 512 : (ih + 1) * 512],
                                start=(ct == 0),
                                stop=(ct == KT - 1),
                            )
                        nc.scalar.activation(
                            out=e_sb[:, jt, ih * 512 : (ih + 1) * 512],
                            in_=pm,
                            func=Exp,
                            scale=SCALE,
                        )
                return e_sb

            def g_gemm1(g_sb):
                h_sb = h_tiles[1]
                for ih in range(NH):
                    for ot in range(KT):
                        pm = ps.tile([128, 512], F32, tag="mm")
                        for kt in range(KT):
                            nc.tensor.matmul(
                                pm,
                                waT_sb[:, kt, ot * 128 : (ot + 1) * 128],
                                h_sb[:, kt, ih * 512 : (ih + 1) * 512],
                                start=(kt == 0),
                                stop=(kt == KT - 1),
                            )
                        nc.vector.tensor_copy(
                            g_sb[:, ot, ih * 512 : (ih + 1) * 512], pm
                        )

            def s_tree(s, e_sb):
                """Reduce e over the 8 j-tiles with 3 wide bf16 DVE adds."""
                t4 = spool.tile([128, 4, N], F16, tag=f"t4_{s}", name=f"t4_{s}")
                t2 = spool.tile([128, 2, N], F16, tag=f"t2_{s}", name=f"t2_{s}")
                t1 = spool.tile([128, N], F16, tag=f"t1_{s}", name=f"t1_{s}")
                nc.vector.tensor_add(t4, e_sb[:, 0:4, :], e_sb[:, 4:8, :])
                nc.vector.tensor_add(t2, t4[:, 0:2, :], t4[:, 2:4, :])
                nc.vector.tensor_add(t1, t2[:, 0, :], t2[:, 1, :])
                return t1

            def s_denom(s, t1):
                """S = ones^T t1 per half; 1/S broadcast to 128 partitions
                via a K=1 ones-matmul (no DRAM bounce)."""
                recipF = spool.tile(
                    [1, N], F32, tag=f"recipF{s}", name=f"recipF{s}"
                )
                recipS = spool.tile(
                    [1, N], F32R, tag=f"recipS{s}", name=f"recipS{s}"
                )
                rSbc = spool.tile([128, N], F32, tag=f"rSbc{s}", name=f"rSbc{s}")
                for ih in range(NH):
                    pS = psg.tile([1, 512], F32, tag="gn", name=f"pS{s}_{ih}")
                    nc.tensor.matmul(
                        pS,
                        ones_col,
                        t1[:, ih * 512 : (ih + 1) * 512],
                        start=True,
                        stop=True,
                    )
                    nc.vector.reciprocal_approx_fast(
                        out=recipF[:, ih * 512 : (ih + 1) * 512], in_=pS
                    )
                    nc.vector.tensor_copy(
                        recipS[:, ih * 512 : (ih + 1) * 512],
                        recipF[:, ih * 512 : (ih + 1) * 512],
                    )
                for ih in range(NH):
                    pb = psg.tile([128, 512], F32, tag="gn", name=f"pb{s}_{ih}")
                    nc.tensor.matmul(
                        pb,
                        ones_row,
                        recipS[:, ih * 512 : (ih + 1) * 512],
                        start=True,
                        stop=True,
                    )
                    nc.vector.tensor_copy(rSbc[:, ih * 512 : (ih + 1) * 512], pb)
                return rSbc

            def o_mm(s, vT_sb, e_sb, gi):
                """The 8-matmul accumulation for one output tile."""
                ct, ih = gi // NH, gi % NH
                pool, tag = (ps, "mm") if gi % 2 == 0 else (psA, "gA")
                pm = pool.tile([128, 512], F32, tag=tag, name=f"o{s}_{gi}")
                for jt in range(NT):
                    nc.tensor.matmul(
                        pm,
                        vT_sb[:, jt, ct * 128 : (ct + 1) * 128],
                        e_sb[:, jt, ih * 512 : (ih + 1) * 512],
                        start=(jt == 0),
                        stop=(jt == NT - 1),
                    )
                return pm

            def o_evac(s, pm, rSbc, gi, dma_eng, gp_add=False, split=1):
                """Normalize by 1/S, add residual in place into the (now
                dead) x tile, stream to DRAM.  split>1 pipelines the
                mul/add/DMA in free-dim slices to shorten the serial
                tail after the last matmul."""
                ct, ih = gi // NH, gi % NH
                x_sb = x_tiles[s]
                t = opool.tile([128, 512], F32, tag="onorm")
                w = 512 // split
                for p in range(split):
                    sl = slice(ih * 512 + p * w, ih * 512 + (p + 1) * w)
                    tl = slice(p * w, (p + 1) * w)
                    nc.vector.tensor_mul(t[:, tl], pm[:, tl], rSbc[:, sl])
                    aeng = nc.gpsimd if gp_add else nc.vector
                    aeng.tensor_add(x_sb[:, ct, sl], t[:, tl], x_sb[:, ct, sl])
                    dma_eng.dma_start(
                        out=y_ext.ap()[s, ct * 128 : (ct + 1) * 128, sl],
                        in_=x_sb[:, ct, sl],
                    )

            def o_group(s, vT_sb, e_sb, rSbc, gi, dma_eng, gp_add=False):
                pm = o_mm(s, vT_sb, e_sb, gi)
                o_evac(s, pm, rSbc, gi, dma_eng, gp_add)

            # SBUF tiles for the attention intermediates
            g0 = gpool.tile([128, KT, N], F16, tag="g0", name="g0")
            g1 = gpool.tile([128, KT, N], F16, tag="g1", name="g1")
            vT0 = vpool.tile([128, NT, C], F16, tag="vT0", name="vT0")
            vT1 = vpool.tile([128, NT, C], F16, tag="vT1", name="vT1")
            e0 = epool.tile([128, NT, N], F16, tag="e0", name="e0")
            e1 = epool.tile([128, NT, N], F16, tag="e1", name="e1")

            # ---- interleaved two-sample schedule (PE FIFO order) ----
            head0(g0)               # gn0 per-kt + g0 sweeps
            stat1 = []
            pv0(vT0, stat1)         # pv0 + gn1 bn_stats interleaved on DVE
            gn1_chain(stat1)        # tiny MMs land between pv0 and scores0
            scores(0, g0, e0)
            g_gemm1(g1)
            t1_0 = s_tree(0, e0)    # DVE during g1
            rS0 = s_denom(0, t1_0)  # PE-broadcast, ready long before o0
            pv1(vT1)
            for gi in range(8):
                o_group(0, vT0, e0, rS0, gi,
                        nc.sync if gi % 2 == 0 else nc.scalar,
                        gp_add=(gi % 2 == 1 and gi < 6))
            scores(1, g1, e1)
            t1_1 = s_tree(1, e1)
            # first two o1 groups' matmuls cover the e1 add-tree latency
            # before the S1 denominator matmuls enter the PE queue
            pm_a = o_mm(1, vT1, e1, 0)
            pm_b = o_mm(1, vT1, e1, 1)
            rS1 = s_denom(1, t1_1)
            o_evac(1, pm_a, rS1, 0, nc.sync)
            o_evac(1, pm_b, rS1, 1, nc.scalar, gp_add=True)
            for gi in range(2, 6):
                o_group(1, vT1, e1, rS1, gi,
                        nc.sync if gi % 2 == 0 else nc.scalar,
                        gp_add=(gi == 3))
            pm_6 = o_mm(1, vT1, e1, 6)
            o_evac(1, pm_6, rS1, 6, nc.sync, split=2)
            pm_7 = o_mm(1, vT1, e1, 7)
            o_evac(1, pm_7, rS1, 7, nc.scalar, split=2)

    nc.compile()
    return nc


def _get_nc():
    if "nc" not in _BUILD_CACHE:
        _BUILD_CACHE["nc"] = _build()
    return _BUILD_CACHE["nc"]


def kernel(x, norm_w, norm_b, qkv_w, qkv_b, proj_w, proj_b, _trace=False):
    global LAST_RESULT

    x = np.asarray(x, dtype=np.float32).reshape(B, C, N)
    norm_w = np.asarray(norm_w, dtype=np.float64)
    norm_b = np.asarray(norm_b, dtype=np.float64)
    qkv_w = np.asarray(qkv_w, dtype=np.float64)
    qkv_b = np.asarray(qkv_b, dtype=np.float64)
    proj_w = np.asarray(proj_w, dtype=np.float64)
    proj_b = np.asarray(proj_b, dtype=np.float64)

    # fold norm affine + proj into the weights (exact, in float64)
    Wq = qkv_w[:C] * norm_w[None, :]
    Wk = qkv_w[C : 2 * C] * norm_w[None, :]
    Wpv = proj_w @ (qkv_w[2 * C :] * norm_w[None, :])
    cq = qkv_w[:C] @ norm_b + qkv_b[:C]          # q bias (k bias dropped)
    b_out = proj_w @ (qkv_w[2 * C :] @ norm_b + qkv_b[2 * C :]) + proj_b
    # scores = (Wq h + cq)^T (Wk h) = h^T A h + (Wk^T cq)^T h.  The rank-1
    # r-term is exactly zero for this problem (norm_b = qkv_b = 0).
    r = Wk.T @ cq
    assert np.abs(r).max() < 1e-9, "nonzero q-bias: single-GEMM scores invalid"
    A = Wq.T @ Wk

    nc = _get_nc()

    waT = np.ascontiguousarray(A.T.astype(_BF16))
    wpvT = np.ascontiguousarray(Wpv.T.astype(_BF16))
    bout_bc = np.ascontiguousarray(
        np.broadcast_to(b_out.astype(_BF16), (128, C))
    )
    ind16 = np.zeros((128, 8), dtype=np.float32)
    for p in range(128):
        ind16[p, p // GS] = 1.0
    ind16T = np.ascontiguousarray(ind16.T)

    shared = {
        "waT": waT,
        "wpvT": wpvT,
        "bout_bc": bout_bc,
        "ind16": ind16,
        "ind16T": ind16T,
        "ones16": np.ones((128, 1), dtype=_BF16),
    }
    in_maps = [
        {"x": np.ascontiguousarray(x[c * SPC : (c + 1) * SPC]), **shared}
        for c in range(NCORES)
    ]
    res = run_bass_kernel_spmd(nc, in_maps, list(range(NCORES)), trace=_trace)
    LAST_RESULT = res
    out = np.concatenate([res.results[i]["y"] for i in range(NCORES)], axis=0)
    return out.reshape(B, C, H, W)


# revision 34
# speedup vs baseline: 1.0245x; 1.0245x over previous
"""AttnBlock (GroupNorm + 1x1-conv QKV + single-head spatial attention + proj
+ residual) on 8 Trainium2 NeuronCores.

Sharding: pure data-parallel over batch — 16 samples / 8 cores = 2 samples per
core; weights broadcast. No collectives; gather on host.

Key optimizations over the 133.6us two-GEMM version:
  - q/k GEMMs replaced by ONE GEMM: softmax is over j, so
    scores = (Wq h)^T (Wk h) = h^T (Wq^T Wk) h = h^T g with g = A h and
    A = Wq^T Wk precomputed on host (f64). Removes 1/7 of the PE work
    and half the QK PSUM->SBUF copies. The q-bias cross term
    (Wk^T cq)^T h is exactly zero here (norm_b = qkv_b = 0; asserted).
  - 1/S broadcast via a K=1 ones-matmul on the PE (~0.2us) instead of
    the DRAM bounce (~6us of exposed HBM round-trip latency).
  - head: bn_stats run per 512KB x chunk as it lands; filler matmuls on
    a memset tile keep the PE HAM-warm through the DMA window so the
    body runs at 2.4GHz from the first real MM. Every strict-FIFO
    engine queue is emitted in expected-ready order (anything else
    head-of-line blocks the queue on a DMA).
  - gn1 emitted between pv0 and scores0 with its DVE stats interleaved
    into the pv0-evac stream, so its Sqrt lands on ACT before the
    exp0 burst: 3 ACT table loads total (exp, sqrt, exp), all hidden.
  - o-group PSUMs alternate between two pools (7 banks effective) so
    the o1 evacs (gated on the S1 chain) never starve the PE of banks;
    the first two o1 groups are emitted before the S1 denominator
    matmuls to cover the e1 add-tree latency.
  - all matmul operands bf16 (1 cycle/row), PSUM accumulation fp32;
    proj folded into the v weights (W_pv = proj_w @ W_v) with its bias
    riding the vT tiles; norm affine folded into the QKV weights.

Per-core formulation (per sample, C=512 channels, N=1024 spatial):
  xn  = (x - mu_g) * rstd_g        (bn_stats + tiny indicator matmuls)
  g   = A' xn                      (C on partitions, spatial free)
  vT  = xn^T Wpv'^T + b_out        (spatial on partitions, C free)
  e   = exp(xn^T g * C^-0.5)       (logits O(1); no max-subtraction)
  S   = ones^T tree(e)
  y   = x + (vT^T e) * (1/S)
"""

import numpy as np
import ml_dtypes
_BF16 = ml_dtypes.bfloat16

import concourse.bass as bass
import concourse.tile as tile
from concourse import bacc, mybir
from concourse.bass_utils import run_bass_kernel_spmd

B, C, H, W = 16, 512, 32, 32
N = H * W              # 1024 spatial positions
G = 32                 # groups
GS = C // G            # 16 channels per group
NCORES = 8
SPC = B // NCORES      # samples per core
EPS = 1e-6
SCALE = float(C) ** -0.5
KT = C // 128          # 4 channel tiles of 128
NT = N // 128          # 8 spatial tiles of 128
NH = N // 512          # 2 free-dim halves of 512

F32 = mybir.dt.float32
F32R = mybir.dt.float32r
F16 = mybir.dt.bfloat16

_BUILD_CACHE = {}
LAST_RESULT = None  # BassKernelResults of the most recent run (for test harness)


def _build():
    nc = bacc.Bacc("TRN2", target_bir_lowering=False, debug=False)

    x_ext = nc.declare_dram_parameter("x", [SPC, C, N], F32, isOutput=False)
    waT_ext = nc.declare_dram_parameter("waT", [C, C], F16, isOutput=False)
    wpvT_ext = nc.declare_dram_parameter("wpvT", [C, C], F16, isOutput=False)
    boutbc_ext = nc.declare_dram_parameter("bout_bc", [128, C], F16, isOutput=False)
    ind_ext = nc.declare_dram_parameter("ind16", [128, 8], F32, isOutput=False)
    indT_ext = nc.declare_dram_parameter("ind16T", [8, 128], F32, isOutput=False)
    ones_ext = nc.declare_dram_parameter("ones16", [128, 1], F16, isOutput=False)
    y_ext = nc.declare_dram_parameter("y", [SPC, C, N], F32, isOutput=True)

    Identity = mybir.ActivationFunctionType.Identity
    Copy = mybir.ActivationFunctionType.Copy
    Exp = mybir.ActivationFunctionType.Exp
    Sqrt = mybir.ActivationFunctionType.Sqrt
    mult = mybir.AluOpType.mult
    add = mybir.AluOpType.add

    with tile.TileContext(nc) as tc:
        with (
            tc.tile_pool(name="wpool", bufs=1) as wpool,
            tc.tile_pool(name="cpool", bufs=1) as cpool,
            tc.tile_pool(name="xpool", bufs=2) as xpool,
            tc.tile_pool(name="hpool", bufs=1) as hpool,
            tc.tile_pool(name="gpool", bufs=1) as gpool,
            tc.tile_pool(name="vpool", bufs=1) as vpool,
            tc.tile_pool(name="epool", bufs=1) as epool,
            tc.tile_pool(name="opool", bufs=4) as opool,
            tc.tile_pool(name="gnpool", bufs=2) as gnpool,
            tc.tile_pool(name="spool", bufs=1) as spool,
            tc.tile_pool(name="psA", bufs=4, space="PSUM") as psA,
            tc.tile_pool(name="ps", bufs=3, space="PSUM") as ps,
            tc.tile_pool(name="psg", bufs=1, space="PSUM") as psg,
        ):
            # ---- DMA plan (proven baseline ring layout): x0 first on both
            # HWDGE rings, small F32R constants on sync, then x1, then the
            # A weights on sync; wpv + remaining constants on SWDGE. ----
            x_tiles = [
                xpool.tile([128, KT, N], F32, tag="x", name=f"x_sb{s}")
                for s in range(SPC)
            ]
            # x0 in 256KB half-chunks so the per-transfer completion
            # semaphores pipeline with the bn_stats consumers
            for kt in range(KT):
                eng = nc.sync if kt % 2 == 0 else nc.scalar
                for sg in range(2):
                    eng.dma_start(
                        out=x_tiles[0][:, kt, sg * 512 : (sg + 1) * 512],
                        in_=x_ext.ap()[
                            0, kt * 128 : (kt + 1) * 128, sg * 512 : (sg + 1) * 512
                        ],
                    )
            ind_sb = cpool.tile([128, 8], F32)
            nc.sync.dma_start(out=ind_sb, in_=ind_ext.ap())
            indT_sb = cpool.tile([8, 128], F32)
            nc.sync.dma_start(out=indT_sb, in_=indT_ext.ap())

            # Everything below is floored: queued transfers round-robin
            # with x0's packets at the SDMA level (different logical
            # queues), so an early issue steals a share of the pipe from
            # the latency-critical x0 stream.
            waT_sb = wpool.tile([128, KT, C], F16)
            wpv_sb = wpool.tile([128, KT, C], F16)
            with tc.tile_wait_until(0.016):
                for kt in range(KT):
                    nc.sync.dma_start(
                        out=waT_sb[:, kt, :],
                        in_=waT_ext.ap()[kt * 128 : (kt + 1) * 128, :],
                    )
                for kt in range(KT):
                    nc.scalar.dma_start(
                        out=wpv_sb[:, kt, :],
                        in_=wpvT_ext.ap()[kt * 128 : (kt + 1) * 128, :],
                    )
            with tc.tile_wait_until(0.018):
                for kt in range(KT):
                    eng = nc.scalar if kt % 2 == 0 else nc.sync
                    eng.dma_start(
                        out=x_tiles[1][:, kt, :],
                        in_=x_ext.ap()[1, kt * 128 : (kt + 1) * 128, :],
                    )
            bout_bc = cpool.tile([128, C], F16)
            nc.gpsimd.dma_start(out=bout_bc, in_=boutbc_ext.ap())
            ones_col = cpool.tile([128, 1], F16)
            nc.gpsimd.dma_start(out=ones_col, in_=ones_ext.ap())

            eps_sb = cpool.tile([8, 1], F32)
            nc.vector.memset(eps_sb, EPS)
            ones_rf = cpool.tile([1, 128], F32)
            nc.vector.memset(ones_rf, 1.0)
            ones_row = cpool.tile([1, 128], F32R)
            nc.vector.tensor_copy(ones_row, ones_rf)
            warm_sb = cpool.tile([8, 1], F32)
            # ACT table warmups: Exp first, Sqrt second, so the sqrt set
            # is resident for both gn chains; one hidden reload to exp at
            # the start of the scores0 burst.
            nc.scalar.activation(out=warm_sb, in_=eps_sb, func=Exp)
            nc.scalar.activation(out=warm_sb, in_=eps_sb, func=Sqrt)

            # ---- PE warmup fillers: keep HAM busy ~8->18us. ----
            warm_mm = cpool.tile([128, 512], F16)
            nc.vector.memset(warm_mm, 0.0)
            _fill_n = [0]

            def fill(n):
                """HAM-keepalive filler matmuls (memset operand, dead PSUM)."""
                for _ in range(n):
                    i = _fill_n[0]
                    _fill_n[0] += 1
                    pw = ps.tile([128, 512], F32, tag="mm", name=f"warm{i}")
                    nc.tensor.matmul(
                        pw, warm_mm[:, 0:128], warm_mm, start=True, stop=True
                    )

            fill(38)

            h_tiles = [
                hpool.tile([128, KT, N], F16, tag=f"h{s}", name=f"h{s}")
                for s in range(SPC)
            ]

            def gn_stats(s):
                """bn_stats per chunk (DVE, paced by the x DMA); batched
                slab chain -> mr (8,KT,2) [-mean*rstd, rstd]."""
                x_sb = x_tiles[s]
                mv = gnpool.tile([128, KT, 2], F32, tag="mv", name=f"mv{s}")
                stat_tiles = []
                for kt in range(KT):
                    stats = gnpool.tile(
                        [128, 2, 6], F32, tag=f"stats{kt}", name=f"stats{s}_{kt}"
                    )
                    stat_tiles.append(stats)
                    for sg in range(2):
                        nc.vector.bn_stats(
                            out=stats[:, sg, :],
                            in_=x_sb[:, kt, sg * 512 : (sg + 1) * 512],
                        )
                for kt in range(KT):
                    nc.vector.bn_aggr(out=mv[:, kt, :], in_=stat_tiles[kt])
                return gn_chain(s, mv, nc.vector)

            def gn_chain(s, mv, eng):
                """eng handles the pure-SBUF slab algebra (nc.vector for
                sample 0 where DVE is free; nc.gpsimd for sample 1 so the
                chain doesn't contend with the pv0/scores0 DVE evacs).
                PSUM reads and the reciprocal stay on DVE.  The indicator
                matmuls run in plain fp32 (tiny free dims) so no F32R
                casts are needed and mv feeds the matmul directly."""
                # in place: mv[...,1] = E[x^2] = var + mean^2
                tmp = gnpool.tile([128, KT], F32, tag="tmp", name=f"tmp{s}")
                eng.tensor_mul(tmp, mv[:, :, 0], mv[:, :, 0])
                eng.tensor_add(mv[:, :, 1], mv[:, :, 1], tmp)
                ps_gs = psg.tile([8, KT, 2], F32, tag="gn", name=f"ps_gs{s}")
                for kt in range(KT):
                    nc.tensor.matmul(
                        ps_gs[:, kt, :], ind_sb, mv[:, kt, :], start=True, stop=True
                    )
                if s == 0:
                    fill(6)  # keep HAM warm while the tiny chain computes
                gs = gnpool.tile([8, KT, 2], F32, tag="gs", name=f"gs{s}")
                nc.vector.tensor_scalar_mul(gs, ps_gs, 1.0 / GS)
                msq = gnpool.tile([8, KT], F32, tag="msq", name=f"msq{s}")
                eng.tensor_mul(msq, gs[:, :, 0], gs[:, :, 0])
                eng.tensor_sub(gs[:, :, 1], gs[:, :, 1], msq)
                nc.scalar.activation(
                    out=gs[:, :, 1], in_=gs[:, :, 1], func=Sqrt, bias=eps_sb
                )
                nc.vector.reciprocal(gs[:, :, 1], gs[:, :, 1])
                eng.tensor_mul(gs[:, :, 0], gs[:, :, 0], gs[:, :, 1])
                eng.tensor_scalar_mul(gs[:, :, 0], gs[:, :, 0], -1.0)
                return gs

            def gn_apply(s, mr, engines):
                """Broadcast stats to channels (one matmul for all kt), apply
                (x-mu)*rstd -> h bf16."""
                x_sb = x_tiles[s]
                ps_bc = psg.tile([128, KT, 2], F32, tag="gn", name=f"ps_bc{s}")
                nc.tensor.matmul(ps_bc, indT_sb, mr, start=True, stop=True)
                scb = gnpool.tile([128, KT, 2], F32, tag="scb", name=f"scb{s}")
                nc.vector.tensor_copy(scb, ps_bc)
                for kt in range(KT):
                    if engines[kt] == "v":
                        nc.vector.tensor_scalar(
                            out=h_tiles[s][:, kt, :],
                            in0=x_sb[:, kt, :],
                            scalar1=scb[:, kt, 1:2],
                            scalar2=scb[:, kt, 0:1],
                            op0=mult,
                            op1=add,
                        )
                    else:
                        nc.scalar.activation(
                            out=h_tiles[s][:, kt, :], in_=x_sb[:, kt, :],
                            func=Identity, scale=scb[:, kt, 1:2],
                            bias=scb[:, kt, 0:1],
                        )

            def head0(g_sb):
                """Sample-0: gn (batched, DMA-paced bn_stats), apply in
                ih-halves (the g ih0 sweep only needs the first halves),
                then g = A' h kt-inner over the ps rotation."""
                mr0 = gn_stats(0)
                ps_bc = psg.tile([128, KT, 2], F32, tag="gn", name="ps_bc0")
                nc.tensor.matmul(ps_bc, indT_sb, mr0, start=True, stop=True)
                fill(4)
                scb = gnpool.tile([128, KT, 2], F32, tag="scb", name="scb0")
                nc.vector.tensor_copy(scb, ps_bc)
                x_sb = x_tiles[0]
                h_sb = h_tiles[0]

                def apply_half(kt, ih, eng):
                    sl = slice(ih * 512, (ih + 1) * 512)
                    if eng == "v":
                        nc.vector.tensor_scalar(
                            out=h_sb[:, kt, sl], in0=x_sb[:, kt, sl],
                            scalar1=scb[:, kt, 1:2], scalar2=scb[:, kt, 0:1],
                            op0=mult, op1=add,
                        )
                    else:
                        nc.scalar.activation(
                            out=h_sb[:, kt, sl], in_=x_sb[:, kt, sl],
                            func=Identity, scale=scb[:, kt, 1:2],
                            bias=scb[:, kt, 0:1],
                        )

                # ih0: kt-OUTER over 4 psA banks so the g matmul cadence
                # (4 MMs ~0.86us per kt) pipelines with the apply cadence
                # (one per-kt half-apply every ~0.5-0.8us, v/a split)
                for kt in range(KT):
                    apply_half(kt, 0, "vava"[kt])
                pms0 = [
                    psA.tile([128, 512], F32, tag="gA", name=f"g0h_{ot}")
                    for ot in range(KT)
                ]
                for kt in range(KT):
                    for ot in range(KT):
                        nc.tensor.matmul(
                            pms0[ot],
                            waT_sb[:, kt, ot * 128 : (ot + 1) * 128],
                            h_sb[:, kt, 0:512],
                            start=(kt == 0),
                            stop=(kt == KT - 1),
                        )
                for kt in range(KT):
                    apply_half(kt, 1, "vava"[kt])
                for ot in range(KT):
                    dest = g_sb[:, ot, 0:512]
                    if ot % 2 == 0:
                        nc.scalar.activation(out=dest, in_=pms0[ot], func=Copy)
                    else:
                        nc.vector.tensor_copy(dest, pms0[ot])
                # ih1: plain kt-inner over the ps rotation (h1 ready)
                for ot in range(KT):
                    pm = ps.tile([128, 512], F32, tag="mm")
                    for kt in range(KT):
                        nc.tensor.matmul(
                            pm,
                            waT_sb[:, kt, ot * 128 : (ot + 1) * 128],
                            h_sb[:, kt, 512:1024],
                            start=(kt == 0),
                            stop=(kt == KT - 1),
                        )
                    dest = g_sb[:, ot, 512:1024]
                    if ot % 2 == 0:
                        nc.scalar.activation(out=dest, in_=pm, func=Copy)
                    else:
                        nc.vector.tensor_copy(dest, pm)

            def gn1_stats_kt(kt, stat_tiles):
                stats = gnpool.tile(
                    [128, 2, 6], F32, tag=f"stats{kt}", name=f"stats1_{kt}"
                )
                stat_tiles.append(stats)
                for sg in range(2):
                    nc.vector.bn_stats(
                        out=stats[:, sg, :],
                        in_=x_tiles[1][:, kt, sg * 512 : (sg + 1) * 512],
                    )

            def gn1_chain(stat_tiles):
                """Batched chain + apply for sample 1; apply on DVE since
                ACT is saturated by exp0."""
                mv = gnpool.tile([128, KT, 2], F32, tag="mv1", name="mv1")
                for kt in range(KT):
                    nc.vector.bn_aggr(out=mv[:, kt, :], in_=stat_tiles[kt])
                mr = gn_chain(1, mv, nc.gpsimd)
                gn_apply(1, mr, "vvvv")

            def pv0(vT_sb, stat1_tiles):
                """vT0 = h0^T Wpv'^T + b_out.  nt0-3 rotate in the ps pool
                (their DVE adds run first so scores0 gets banks on time);
                nt4-7 use the psA banks (their adds are interleaved with
                gn1's DVE stats and may run late — harmless)."""
                h_sb = h_tiles[0]
                pms = {}
                for nt in range(NT):
                    pool = ps if nt < 4 else psA
                    pm = pool.tile([128, 512], F32, tag="mm" if nt < 4 else "gA",
                                   name=f"pv0_{nt}")
                    pms[nt] = pm
                    for kt in range(KT):
                        nc.tensor.matmul(
                            pm,
                            h_sb[:, kt, nt * 128 : (nt + 1) * 128],
                            wpv_sb[:, kt, :],
                            start=(kt == 0),
                            stop=(kt == KT - 1),
                        )
                for nt in range(4):
                    nc.vector.tensor_add(vT_sb[:, nt, :], pms[nt], bout_bc)
                # gn1's DVE stats, floored so the scheduler cannot slot them
                # between gn0's chain ops (strict-FIFO head-of-line risk)
                with tc.tile_wait_until(0.026):
                    for kt in range(KT):
                        gn1_stats_kt(kt, stat1_tiles)
                for nt in range(4, NT):
                    nc.vector.tensor_add(vT_sb[:, nt, :], pms[nt], bout_bc)

            def pv1(vT_sb):
                h_sb = h_tiles[1]
                for nt in range(NT):
                    pm = ps.tile([128, 512], F32, tag="mm")
                    for kt in range(KT):
                        nc.tensor.matmul(
                            pm,
                            h_sb[:, kt, nt * 128 : (nt + 1) * 128],
                            wpv_sb[:, kt, :],
                            start=(kt == 0),
                            stop=(kt == KT - 1),
                        )
                    nc.vector.tensor_add(vT_sb[:, nt, :], pm, bout_bc)

            def scores(s, g_sb, e_sb):
                """e = exp(h^T g * C^-0.5); keys on partitions."""
                h_sb = h_tiles[s]
                for ih in range(NH):
                    for jt in range(NT):
                        pm = ps.tile([128, 512], F32, tag="mm")
                        for ct in range(KT):
                            nc.tensor.matmul(
                                pm,
                                g_sb[:, ct, jt * 128 : (jt + 1) * 128],
                                h_sb[:, ct, ih * 512 : (ih + 1) * 512],
                                start=(ct == 0),
                                stop=(ct == KT - 1),
                            )
                        nc.scalar.activation(
                            out=e_sb[:, jt, ih * 512 : (ih + 1) * 512],
                            in_=pm,
                            func=Exp,
                            scale=SCALE,
                        )
                return e_sb

            def g_gemm1(g_sb):
                h_sb = h_tiles[1]
                for ih in range(NH):
                    for ot in range(KT):
                        pm = ps.tile([128, 512], F32, tag="mm")
                        for kt in range(KT):
                            nc.tensor.matmul(
                                pm,
                                waT_sb[:, kt, ot * 128 : (ot + 1) * 128],
                                h_sb[:, kt, ih * 512 : (ih + 1) * 512],
                                start=(kt == 0),
                                stop=(kt == KT - 1),
                            )
                        nc.vector.tensor_copy(
                            g_sb[:, ot, ih * 512 : (ih + 1) * 512], pm
                        )

            def s_tree(s, e_sb):
                """Reduce e over the 8 j-tiles with 3 wide bf16 DVE adds."""
                t4 = spool.tile([128, 4, N], F16, tag=f"t4_{s}", name=f"t4_{s}")
                t2 = spool.tile([128, 2, N], F16, tag=f"t2_{s}", name=f"t2_{s}")
                t1 = spool.tile([128, N], F16, tag=f"t1_{s}", name=f"t1_{s}")
                nc.vector.tensor_add(t4, e_sb[:, 0:4, :], e_sb[:, 4:8, :])
                nc.vector.tensor_add(t2, t4[:, 0:2, :], t4[:, 2:4, :])
                nc.vector.tensor_add(t1, t2[:, 0, :], t2[:, 1, :])
                return t1

            def s_denom(s, t1):
                """S = ones^T t1 per half; 1/S broadcast to 128 partitions
                via a K=1 ones-matmul (no DRAM bounce)."""
                recipF = spool.tile(
                    [1, N], F32, tag=f"recipF{s}", name=f"recipF{s}"
                )
                recipS = spool.tile(
                    [1, N], F32R, tag=f"recipS{s}", name=f"recipS{s}"
                )
                rSbc = spool.tile([128, N], F32, tag=f"rSbc{s}", name=f"rSbc{s}")
                for ih in range(NH):
                    pS = psg.tile([1, 512], F32, tag="gn", name=f"pS{s}_{ih}")
                    nc.tensor.matmul(
                        pS,
                        ones_col,
                        t1[:, ih * 512 : (ih + 1) * 512],
                        start=True,
                        stop=True,
                    )
                    nc.vector.reciprocal_approx_fast(
                        out=recipF[:, ih * 512 : (ih + 1) * 512], in_=pS
                    )
                    nc.vector.tensor_copy(
                        recipS[:, ih * 512 : (ih + 1) * 512],
                        recipF[:, ih * 512 : (ih + 1) * 512],
                    )
                for ih in range(NH):
                    pb = psg.tile([128, 512], F32, tag="gn", name=f"pb{s}_{ih}")
                    nc.tensor.matmul(
                        pb,
                        ones_row,
                        recipS[:, ih * 512 : (ih + 1) * 512],
                        start=True,
                        stop=True,
                    )
                    nc.vector.tensor_copy(rSbc[:, ih * 512 : (ih + 1) * 512], pb)
                return rSbc

            def o_mm(s, vT_sb, e_sb, gi):
                """The 8-matmul accumulation for one output tile."""
                ct, ih = gi // NH, gi % NH
                pool, tag = (ps, "mm") if gi % 2 == 0 else (psA, "gA")
                pm = pool.tile([128, 512], F32, tag=tag, name=f"o{s}_{gi}")
                for jt in range(NT):
                    nc.tensor.matmul(
                        pm,
                        vT_sb[:, jt, ct * 128 : (ct + 1) * 128],
                        e_sb[:, jt, ih * 512 : (ih + 1) * 512],
                        start=(jt == 0),
                        stop=(jt == NT - 1),
                    )
                return pm

            def o_evac(s, pm, rSbc, gi, dma_eng, gp_add=False, split=1):
                """Normalize by 1/S, add residual in place into the (now
                dead) x tile, stream to DRAM.  split>1 pipelines the
                mul/add/DMA in free-dim slices to shorten the serial
                tail after the last matmul."""
                ct, ih = gi // NH, gi % NH
                x_sb = x_tiles[s]
                t = opool.tile([128, 512], F32, tag="onorm")
                w = 512 // split
                for p in range(split):
                    sl = slice(ih * 512 + p * w, ih * 512 + (p + 1) * w)
                    tl = slice(p * w, (p + 1) * w)
                    nc.vector.tensor_mul(t[:, tl], pm[:, tl], rSbc[:, sl])
                    aeng = nc.gpsimd if gp_add else nc.vector
                    aeng.tensor_add(x_sb[:, ct, sl], t[:, tl], x_sb[:, ct, sl])
                    dma_eng.dma_start(
                        out=y_ext.ap()[s, ct * 128 : (ct + 1) * 128, sl],
                        in_=x_sb[:, ct, sl],
                    )

            def o_group(s, vT_sb, e_sb, rSbc, gi, dma_eng, gp_add=False):
                pm = o_mm(s, vT_sb, e_sb, gi)
                o_evac(s, pm, rSbc, gi, dma_eng, gp_add)

            # SBUF tiles for the attention intermediates
            g0 = gpool.tile([128, KT, N], F16, tag="g0", name="g0")
            g1 = gpool.tile([128, KT, N], F16, tag="g1", name="g1")
            vT0 = vpool.tile([128, NT, C], F16, tag="vT0", name="vT0")
            vT1 = vpool.tile([128, NT, C], F16, tag="vT1", name="vT1")
            e0 = epool.tile([128, NT, N], F16, tag="e0", name="e0")
            e1 = epool.tile([128, NT, N], F16, tag="e1", name="e1")

            # ---- interleaved two-sample schedule (PE FIFO order) ----
            head0(g0)               # gn0 per-kt + g0 sweeps
            stat1 = []
            pv0(vT0, stat1)         # pv0 + gn1 bn_stats interleaved on DVE
            gn1_chain(stat1)        # tiny MMs land between pv0 and scores0
            scores(0, g0, e0)
            g_gemm1(g1)
            t1_0 = s_tree(0, e0)    # DVE during g1
            rS0 = s_denom(0, t1_0)  # PE-broadcast, ready long before o0
            pv1(vT1)
            for gi in range(8):
                o_group(0, vT0, e0, rS0, gi,
                        nc.sync if gi % 2 == 0 else nc.scalar,
                        gp_add=(gi % 2 == 1 and gi < 6))
            scores(1, g1, e1)
            t1_1 = s_tree(1, e1)
            # first two o1 groups' matmuls cover the e1 add-tree latency
            # before the S1 denominator matmuls enter the PE queue
            pm_a = o_mm(1, vT1, e1, 0)
            pm_b = o_mm(1, vT1, e1, 1)
            rS1 = s_denom(1, t1_1)
            o_evac(1, pm_a, rS1, 0, nc.sync)
            o_evac(1, pm_b, rS1, 1, nc.scalar, gp_add=True)
            for gi in range(2, 6):
                o_group(1, vT1, e1, rS1, gi,
                        nc.sync if gi % 2 == 0 else nc.scalar,
                        gp_add=(gi == 3))
            pm_6 = o_mm(1, vT1, e1, 6)
            o_evac(1, pm_6, rS1, 6, nc.sync, split=2)
            pm_7 = o_mm(1, vT1, e1, 7)
            o_evac(1, pm_7, rS1, 7, nc.scalar, split=2)

    nc.compile()
    return nc


def _get_nc():
    if "nc" not in _BUILD_CACHE:
        _BUILD_CACHE["nc"] = _build()
    return _BUILD_CACHE["nc"]


def kernel(x, norm_w, norm_b, qkv_w, qkv_b, proj_w, proj_b, _trace=False):
    global LAST_RESULT

    x = np.asarray(x, dtype=np.float32).reshape(B, C, N)
    norm_w = np.asarray(norm_w, dtype=np.float64)
    norm_b = np.asarray(norm_b, dtype=np.float64)
    qkv_w = np.asarray(qkv_w, dtype=np.float64)
    qkv_b = np.asarray(qkv_b, dtype=np.float64)
    proj_w = np.asarray(proj_w, dtype=np.float64)
    proj_b = np.asarray(proj_b, dtype=np.float64)

    # fold norm affine + proj into the weights (exact, in float64)
    Wq = qkv_w[:C] * norm_w[None, :]
    Wk = qkv_w[C : 2 * C] * norm_w[None, :]
    Wpv = proj_w @ (qkv_w[2 * C :] * norm_w[None, :])
    cq = qkv_w[:C] @ norm_b + qkv_b[:C]          # q bias (k bias dropped)
    b_out = proj_w @ (qkv_w[2 * C :] @ norm_b + qkv_b[2 * C :]) + proj_b
    # scores = (Wq h + cq)^T (Wk h) = h^T A h + (Wk^T cq)^T h.  The rank-1
    # r-term is exactly zero for this problem (norm_b = qkv_b = 0).
    r = Wk.T @ cq
    assert np.abs(r).max() < 1e-9, "nonzero q-bias: single-GEMM scores invalid"
    A = Wq.T @ Wk

    nc = _get_nc()

    waT = np.ascontiguousarray(A.T.astype(_BF16))
    wpvT = np.ascontiguousarray(Wpv.T.astype(_BF16))
    bout_bc = np.ascontiguousarray(
        np.broadcast_to(b_out.astype(_BF16), (128, C))
    )
    ind16 = np.zeros((128, 8), dtype=np.float32)
    for p in range(128):
        ind16[p, p // GS] = 1.0
    ind16T = np.ascontiguousarray(ind16.T)

    shared = {
        "waT": waT,
        "wpvT": wpvT,
        "bout_bc": bout_bc,
        "ind16": ind16,
        "ind16T": ind16T,
        "ones16": np.ones((128, 1), dtype=_BF16),
    }
    in_maps = [
        {"x": np.ascontiguousarray(x[c * SPC : (c + 1) * SPC]), **shared}
        for c in range(NCORES)
    ]
    res = run_bass_kernel_spmd(nc, in_maps, list(range(NCORES)), trace=_trace)
    LAST_RESULT = res
    out = np.concatenate([res.results[i]["y"] for i in range(NCORES)], axis=0)
    return out.reshape(B, C, H, W)
